# revision 8
# baseline (speedup 1.0000x reference)
import numpy as np

import concourse.bass as bass
import concourse.tile as tile
from concourse import bacc, mybir
from concourse.bass_utils import run_bass_kernel_spmd

B, T, D, R = 4, 2048, 1024, 64
TH = T // 2
NT = TH // 128
ND = D // 128
PCOLS = 448
CUM0 = 192
NCUM = 256

F32 = mybir.dt.float32
F16 = mybir.dt.float16


def build_nc():
    nc = bacc.Bacc(None, target_bir_lowering=False)

    xT = nc.dram_tensor("xT", [D, TH], F16, kind="ExternalInput")
    P = nc.dram_tensor("P", [D, PCOLS], F16, kind="ExternalInput")
    AT = nc.dram_tensor("AT", [128, D], F16, kind="ExternalInput")
    U = nc.dram_tensor("U", [128, 128], F16, kind="ExternalInput")
    EK = nc.dram_tensor("EK", [NT, 128, 128], F16, kind="ExternalInput")
    IDN = nc.dram_tensor("IDN", [128, 128], F16, kind="ExternalInput")
    sxT = nc.dram_tensor("sxT", [128, ND], F16, kind="ExternalInput")
    invc = nc.dram_tensor("invc", [128, NT], F32, kind="ExternalInput")
    outT = nc.dram_tensor("outT", [D, TH], F16, kind="ExternalOutput")

    with tile.TileContext(nc) as tc:
        with tc.tile_pool(name="consts", bufs=1) as consts, \
             tc.tile_pool(name="big", bufs=1) as big, \
             tc.tile_pool(name="outp", bufs=3) as outp, \
             tc.tile_pool(name="ps", bufs=2, space="PSUM") as ps:

            warm_sb = consts.tile([128, 128], F16)
            nc.vector.memset(warm_sb, 0.0)
            warm_ps = ps.tile([128, 128], F32, tag="misc", bufs=1)
            for i in range(34):
                nc.tensor.matmul(warm_ps, warm_sb, warm_sb,
                                 start=True, stop=True)

            xT_sb = big.tile([128, ND, TH], F16)
            P_sb = consts.tile([128, ND, PCOLS], F16)
            xTv = xT.rearrange("(k p) t -> p k t", p=128)
            Pv = P.rearrange("(k p) c -> p k c", p=128)
            for dk in range(ND):
                nc.sync.dma_start(out=xT_sb[:, dk, :], in_=xTv[:, dk, :])
                nc.scalar.dma_start(out=P_sb[:, dk, :], in_=Pv[:, dk, :])
            U_sb = consts.tile([128, 128], F16)
            nc.scalar.dma_start(out=U_sb, in_=U[:, :])
            EK_sb = consts.tile([128, NT, 128], F16)
            nc.scalar.dma_start(out=EK_sb, in_=EK.rearrange("k p c -> p k c"))
            IDN_sb = consts.tile([128, 128], F16)
            nc.scalar.dma_start(out=IDN_sb, in_=IDN[:, :])
            AT_sb = consts.tile([128, D], F16)
            nc.scalar.dma_start(out=AT_sb, in_=AT[:, :])
            sxT_sb = consts.tile([128, ND], F16)
            nc.scalar.dma_start(out=sxT_sb, in_=sxT[:, :])
            invc_sb = consts.tile([128, NT], F32)
            nc.scalar.dma_start(out=invc_sb, in_=invc[:, :])

            xp_sb = big.tile([128, NT, PCOLS], F16)
            cum_sb = big.tile([128, NT, NCUM], F16)
            ci_sb = big.tile([128, NCUM], F16)

            def emit_xp(k):
                xp_ps = ps.tile([128, PCOLS], F32, tag="xp")
                for dk in range(ND):
                    nc.tensor.matmul(xp_ps,
                                     xT_sb[:, dk, k * 128:(k + 1) * 128],
                                     P_sb[:, dk, :],
                                     start=(dk == 0), stop=(dk == ND - 1))
                nc.vector.tensor_copy(xp_sb[:, k, :], xp_ps)

            def emit_cum(k):
                cum_ps = ps.tile([128, NCUM], F32, tag="cum")
                nc.tensor.matmul(cum_ps, U_sb, xp_sb[:, k, CUM0:],
                                 start=True, stop=False)
                prev = ci_sb if k == 0 else cum_sb[:, k - 1, :]
                nc.tensor.matmul(cum_ps, EK_sb[:, k, :], prev,
                                 start=False, stop=True)
                nc.vector.tensor_scalar_mul(cum_sb[:, k, :], cum_ps,
                                            invc_sb[:, k:k + 1])

            emit_xp(0)

            ci_ps = ps.tile([1, NCUM], F32, tag="misc", bufs=1)
            for dk in range(ND):
                nc.tensor.matmul(ci_ps, sxT_sb[:, dk:dk + 1],
                                 P_sb[:, dk, CUM0:],
                                 start=(dk == 0), stop=(dk == ND - 1))
            nc.vector.memset(ci_sb, 0.0)
            nc.vector.tensor_copy(ci_sb[0:1, :], ci_ps[0:1, :])

            G_sb = big.tile([128, NT, 128], F16)
            m2_sb = big.tile([128, NT, 64], F32)

            def emit_ew(lo, hi):
                xps = xp_sb[:, lo:hi, :]
                cms = cum_sb[:, lo:hi, :]
                m2 = m2_sb[:, lo:hi, :]
                g = G_sb[:, lo:hi, :]
                nc.vector.tensor_mul(m2, xps[:, :, 64:128], cms[:, :, 64:128])
                nc.vector.tensor_mul(g[:, :, 0:64], xps[:, :, 0:64],
                                     cms[:, :, 0:64])
                nc.vector.tensor_add(g[:, :, 0:64], g[:, :, 0:64], m2)
                nc.vector.tensor_mul(g[:, :, 64:128], xps[:, :, 128:192],
                                     cms[:, :, 128:192])
                nc.vector.tensor_mul(g[:, :, 64:128], g[:, :, 64:128],
                                     cms[:, :, 192:256])

            GT_sb = big.tile([128, TH], F16)

            def emit_tp(k):
                gt_ps = ps.tile([128, 128], F16, tag="misc", bufs=1)
                nc.tensor.transpose(gt_ps, G_sb[:, k, :], IDN_sb)
                nc.vector.tensor_copy(GT_sb[:, k * 128:(k + 1) * 128], gt_ps)

            def emit_final(n):
                for dk in range(ND):
                    o_ps = ps.tile([128, 512], F32, tag="o", bufs=3)
                    nc.tensor.matmul(o_ps,
                                     AT_sb[:, dk * 128:(dk + 1) * 128],
                                     GT_sb[:, n * 512:(n + 1) * 512],
                                     start=True, stop=True)
                    o_sb = outp.tile([128, 512], F16)
                    if dk % 2 == 0:
                        nc.vector.tensor_copy(o_sb, o_ps)
                    else:
                        nc.scalar.copy(o_sb, o_ps)
                    nc.sync.dma_start(
                        out=outT[dk * 128:(dk + 1) * 128,
                                 n * 512:(n + 1) * 512],
                        in_=o_sb)

            emit_xp(1)
            emit_cum(0)
            emit_xp(2)
            emit_cum(1)
            emit_xp(3)
            emit_cum(2)
            emit_xp(4)
            emit_cum(3)
            emit_xp(5)
            emit_ew(0, 4)
            emit_xp(6)
            emit_cum(4)
            for k in range(4):
                emit_tp(k)
            emit_xp(7)
            emit_cum(5)
            emit_cum(6)
            emit_cum(7)
            emit_ew(4, NT)
            emit_final(0)
            for k in range(4, NT):
                emit_tp(k)
            emit_final(1)

    nc.finalize()
    return nc


_NC = None


def _get_nc():
    global _NC
    if _NC is None:
        _NC = build_nc()
    return _NC


def _fold_weights(WQ, WK, WO, Winv, U_b, V_b, W_b, U_t, V_t, W_t, X_t,
                  alpha_bi, alpha_tri):
    f8 = np.float64
    WQt = WQ.astype(f8).T
    WKt = WK.astype(f8).T
    Winvt = Winv.astype(f8).T
    P = np.concatenate([
        WQt @ V_b.astype(f8),
        float(alpha_bi) * (WQt @ (Winvt @ W_b.astype(f8))),
        WQt @ V_t.astype(f8),
        WKt @ W_b.astype(f8),
        WKt @ (Winvt @ V_b.astype(f8)),
        WKt @ W_t.astype(f8),
        X_t.astype(f8),
    ], axis=1).astype(np.float32)
    A = np.concatenate([
        WO.astype(f8) @ U_b.astype(f8),
        float(alpha_tri) * (WO.astype(f8) @ U_t.astype(f8)),
    ], axis=1).astype(np.float32)
    return P, A


def _make_consts(h):
    U = np.triu(np.ones((128, 128), np.float16))
    EK = np.zeros((NT, 128, 128), np.float16)
    EK[0, 0, :] = 1.0
    for k in range(1, NT):
        EK[k, 127, :] = np.float16(h * TH + k * 128)
    IDN = np.eye(128, dtype=np.float16)
    counts = np.arange(h * TH + 1, (h + 1) * TH + 1, dtype=np.float64)
    invc = np.ascontiguousarray(
        (1.0 / counts).astype(np.float32).reshape(NT, 128).T)
    return U, EK, IDN, invc


def make_in_maps(x, P, A):
    AT = np.ascontiguousarray(A.T.astype(np.float16))
    P16 = P.astype(np.float16)
    in_maps = []
    for core in range(8):
        b, h = core // 2, core % 2
        xTc = np.ascontiguousarray(x[b, h * TH:(h + 1) * TH, :].T
                                   .astype(np.float16))
        if h == 1:
            sx = x[b, :TH, :].sum(axis=0, dtype=np.float64)
        else:
            sx = np.zeros(D, np.float64)
        sxT = np.ascontiguousarray(
            sx.astype(np.float16).reshape(ND, 128).T)
        U, EK, IDN, invc = _make_consts(h)
        in_maps.append(dict(xT=xTc, P=P16, AT=AT, U=U, EK=EK,
                            IDN=IDN, sxT=sxT, invc=invc))
    return in_maps


def kernel(x, WQ, WK, WO, Winv, U_b, V_b, W_b, bias_b,
           U_t, V_t, W_t, X_t, bias_t, alpha_bi, alpha_tri):
    x = np.asarray(x, dtype=np.float32)
    P, A = _fold_weights(WQ, WK, WO, Winv, U_b, V_b, W_b,
                         U_t, V_t, W_t, X_t, alpha_bi, alpha_tri)
    in_maps = make_in_maps(x, P, A)

    res = run_bass_kernel_spmd(_get_nc(), in_maps, core_ids=list(range(8)))

    out = np.empty((B, T, D), np.float32)
    for core in range(8):
        b, h = core // 2, core % 2
        out[b, h * TH:(h + 1) * TH, :] = \
            res.results[core]["outT"].T.astype(np.float32)

    bias_out = ((1.0 + float(alpha_bi)) * np.asarray(bias_b, np.float64)
                + float(alpha_tri) * np.asarray(bias_t, np.float64)) \
        @ np.asarray(WO, np.float64).T
    if np.any(bias_out):
        out += bias_out.astype(np.float32)[None, None, :]
    return out
